# revision 1
# baseline (speedup 1.0000x reference)
"""Trainium2 Bass kernel for nn_CliffordRollingAttention.

Strategy (head-parallel over 8 cores, 2 heads/core):
  - Host pre-transposes x -> xT [D, B*L] bf16, slices/folds weights per core.
  - On-device per core:
      P1: QKV projections on PE in transposed layout [d, l] (stationary =
          weight tiles, moving = xT), bf16 with fp32 PSUM accumulation.
          Per-row sumsq partials for RMS norm via ACT Square + PE ones-reduce.
      P2: 64KB AllReduce of q/k sumsq partials across the 8 cores,
          rsqrt in column-form.
      P3: rms broadcast via K=1 matmul; normalize k; channel-roll score
          mixing folded into a host-built 128x128 matrix (Pm) -> one matmul.
      P4: scores: DVE products qm * k[:, l-s] (seq shift = free-dim offset)
          + PE one-hot-column reduce accumulating [16 shifts, 512 l] in PSUM.
          Max-free softmax (|logit| <= ~4.2): ACT exp, PE ones denom.
      P5: PE-transpose exp rows to row-major [l, 17], normalize attn.
      P6: apply-V row-major: v re-read shifted via DMA-transpose from a
          transposed DRAM copy; fused scalar_tensor_tensor accumulation
          in two bf16 chains of 8 shifts, merged in fp32.
      P7: output projection partial (this core's 256 channels), bf16 out.
  - Host sums the 8 partials in fp32 and adds the output bias.
"""

import numpy as np
import ml_dtypes

import concourse.bass as bass
import concourse.bacc as bacc
import concourse.mybir as mybir
import concourse.tile as tile
from concourse.bass_utils import run_bass_kernel_spmd

BF = ml_dtypes.bfloat16
FP32 = mybir.dt.float32
BF16 = mybir.dt.bfloat16

B, L, D = 2, 4096, 2048
H, DH = 16, 128
NCORES = 8
HPC = H // NCORES          # heads per core = 2
DPC = HPC * DH             # channels per core = 256
N = B * L                  # 8192 rows
EPS = 1e-6
SEQ_SHIFTS = [0, 1, -1, 3, -3, 9, -9, 26, -26, 78, -78, 232, -232, 689, -689, 2048]
CH_SHIFTS = [1, 2, 4, 8]
NS = len(SEQ_SHIFTS)       # 16
CHUNK = 512
NCHUNK = N // CHUNK        # 16
NLT = N // 128             # 64 l-tiles
AluOp = mybir.AluOpType
AF = mybir.ActivationFunctionType


def _wrap_runs(start, length):
    """Split output positions j in [0,length) whose source row is
    b*L + ((start_local + j) mod L), into maximal contiguous source runs.
    `start` is an absolute flattened row index that may be out of range
    within its batch. The batch is determined by the caller; here `start`
    is already batch-local (can be negative or >= L). Returns list of
    (j_offset, src_local_start, run_len)."""
    runs = []
    j = 0
    while j < length:
        src = (start + j) % L
        run = min(length - j, L - src)
        runs.append((j, src, run))
        j += run
    return runs


def _build_program():
    nc = bacc.Bacc(num_devices=NCORES)

    handles = {
        "xT": nc.declare_dram_parameter("xT", [D, N], BF16, isOutput=False),
        "wT": nc.declare_dram_parameter("wT", [D, 6 * 128], BF16, isOutput=False),
        "bias6": nc.declare_dram_parameter("bias6", [128, 6], FP32, isOutput=False),
        "pmT": nc.declare_dram_parameter("pmT", [128, HPC * 128], BF16, isOutput=False),
        "oh": nc.declare_dram_parameter("oh", [128, NS * 16], BF16, isOutput=False),
        "idm": nc.declare_dram_parameter("idm", [128, 128], BF16, isOutput=False),
        "ones_r": nc.declare_dram_parameter("ones_r", [1, 128], FP32, isOutput=False),
        "woT": nc.declare_dram_parameter("woT", [DPC, D], BF16, isOutput=False),
        "outp": nc.declare_dram_parameter("outp", [N, D], BF16, isOutput=True),
    }

    import contextlib
    with tile.TileContext(nc) as tc:
        with contextlib.ExitStack() as ctx:
            _emit_inner(ctx, tc, handles)
    nc.compile()
    return nc


def _emit_inner(ctx, tc, handles):
    nc = tc.nc
    xT = handles["xT"][:]
    wT = handles["wT"][:]
    bias6 = handles["bias6"][:]
    pmT_d = handles["pmT"][:]
    oh_d = handles["oh"][:]
    idm_d = handles["idm"][:]
    ones_r_d = handles["ones_r"][:]
    woT_d = handles["woT"][:]
    outp = handles["outp"][:]

    # ---------------- persistent pools ----------------
    const = ctx.enter_context(tc.tile_pool(name="const", bufs=1))
    big = ctx.enter_context(tc.tile_pool(name="big", bufs=1))
    dram = ctx.enter_context(tc.tile_pool(name="dram", bufs=1, space="DRAM"))

    w_sb = const.tile([128, 16 * 768], BF16)        # 24KB
    bias_sb = const.tile([128, 6], FP32)
    pm_sb = const.tile([128, HPC * 128], BF16)
    oh_sb = const.tile([128, NS * 16], BF16)
    id_sb = const.tile([128, 128], BF16)
    onesr_sb = const.tile([1, 128], FP32)
    eps_sb = const.tile([128, 1], FP32)
    wo_sb = const.tile([128, HPC * D], BF16)        # 8KB: [128, 2*2048] dtile-major

    # wT [2048, 768] -> w_sb[p, k*768+j] = wT[128k+p, j]
    nc.sync.dma_start(w_sb[:].rearrange("p (k j) -> p k j", k=16),
                      wT.rearrange("(k p) j -> p k j", p=128))
    nc.sync.dma_start(bias_sb[:], bias6)
    nc.sync.dma_start(pm_sb[:], pmT_d)
    nc.sync.dma_start(oh_sb[:], oh_d)
    nc.sync.dma_start(id_sb[:], idm_d)
    nc.sync.dma_start(onesr_sb[:], ones_r_d)
    nc.gpsimd.memset(eps_sb[:], EPS)
    # woT [256, 2048] -> [p, dt*2048 + e]
    nc.sync.dma_start(wo_sb[:].rearrange("p (dt e) -> p dt e", dt=2),
                      woT_d.rearrange("(dt p) e -> p dt e", p=128))

    q_raw = big.tile([128, HPC * N], BF16, tag="qraw")   # 32KB  [p, h*N + l]
    k_sb = big.tile([128, HPC * N], BF16)                # 32KB
    qm_sb = big.tile([128, HPC * N], BF16)               # 32KB
    exp_sb = big.tile([16, HPC * N], BF16, tag="qraw")   # 32KB, shares slot with q_raw (disjoint lifetimes)
    attn_all = big.tile([128, NLT * HPC * 16], FP32)     # 4KB   [p, (t*2+h)*16 + i]

    v_dram = dram.tile([N, DPC], BF16)
    ss_dram = dram.tile([2, 2, N // 2], FP32)   # [half, q/k, l-in-half]
    ss_out = dram.tile([2, 2, N // 2], FP32)
    rms_dram = dram.tile([2, N], FP32)

    # ---------------- P2: AllReduce + rsqrt (split into halves) ----------
    def emit_rms_half(hf):
        HN = N // 2
        nc.gpsimd.collective_compute(
            "AllReduce", AluOp.add,
            replica_groups=[list(range(NCORES))],
            ins=[ss_dram[hf].opt()],
            outs=[ss_out[hf].opt()],
        )
        with tc.tile_pool(name=f"p2_{hf}", bufs=1) as p2:
            col = p2.tile([128, 64], FP32, name=f"col_{hf}")
            srt = p2.tile([128, 64], FP32, name=f"srt_{hf}")
            rinv = p2.tile([128, 64], FP32, name=f"rinv_{hf}")
            for r in range(2):
                nc.sync.dma_start(
                    col[:, r * 32:(r + 1) * 32],
                    ss_out[hf, r, :].rearrange("(t p) -> p t", p=128))
            nc.scalar.activation(srt[:], col[:], AF.Sqrt, bias=eps_sb[:],
                                 scale=1.0 / D)
            nc.vector.reciprocal(rinv[:], srt[:])
            for r in range(2):
                nc.sync.dma_start(
                    rms_dram[r, hf * HN:(hf + 1) * HN]
                    .rearrange("(t p) -> p t", p=128),
                    rinv[:, r * 32:(r + 1) * 32])

    # ---------------- P1: projections ----------------
    with tc.tile_pool(name="p1x", bufs=2) as p1x, \
         tc.tile_pool(name="p1ps", bufs=1, space="PSUM") as p1ps, \
         tc.tile_pool(name="p1ss", bufs=1, space="PSUM") as p1ss, \
         tc.tile_pool(name="p1tp", bufs=1, space="PSUM") as p1tp, \
         tc.tile_pool(name="p1sc", bufs=2) as p1sc:
        for c in range(NCHUNK):
            cs = c * CHUNK
            psums = [p1ps.tile([128, CHUNK], FP32, tag=f"proj{m}", name=f"proj{m}_{c}") for m in range(6)]
            xt = p1x.tile([128, 16, CHUNK], BF16, tag="xt")
            nc.sync.dma_start(xt[:],
                              xT[:, cs:cs + CHUNK].rearrange("(k p) l -> p k l", p=128))
            for k in range(16):
                for m in range(6):
                    nc.tensor.matmul(
                        psums[m][:],
                        w_sb[:, k * 768 + 128 * m: k * 768 + 128 * (m + 1)],
                        xt[:, k, :],
                        start=(k == 0), stop=(k == 15),
                    )
            # m order: q0 q1 k0 k1 v0 v1
            vrm = p1sc.tile([128, 4, DPC], BF16, tag="vrm", name=f"vrm_{c}")
            # q: raw evict + squares + sumsq partial
            ssq = p1ss.tile([1, CHUNK], FP32, tag="ss", name=f"ssq_{c}")
            for dt in range(2):
                nc.scalar.activation(
                    q_raw[:, dt * N + cs: dt * N + cs + CHUNK], psums[dt][:],
                    AF.Identity, bias=bias_sb[:, dt:dt + 1])
                sq = p1sc.tile([128, CHUNK], BF16, tag="sq")
                nc.scalar.activation(sq[:], psums[dt][:], AF.Square,
                                     bias=bias_sb[:, dt:dt + 1])
                nc.tensor.matmul(ssq[:], oh_sb[:, 0:1], sq[:],
                                 start=(dt == 0), stop=(dt == 1))
            ssr_q = p1sc.tile([1, CHUNK], FP32, tag="ssrq", name=f"ssrq_{c}")
            nc.scalar.activation(ssr_q[:], ssq[:], AF.Copy)
            hfc, hcs = divmod(cs, N // 2)
            nc.sync.dma_start(ss_dram[hfc, 0:1, hcs:hcs + CHUNK], ssr_q[:])
            # k
            ssk = p1ss.tile([1, CHUNK], FP32, tag="ss", name=f"ssk_{c}")
            for dt in range(2):
                nc.scalar.activation(
                    k_sb[:, dt * N + cs: dt * N + cs + CHUNK], psums[2 + dt][:],
                    AF.Identity, bias=bias_sb[:, 2 + dt:3 + dt])
                sqk = p1sc.tile([128, CHUNK], BF16, tag="sq", name=f"sqk_{c}_{dt}")
                nc.scalar.activation(sqk[:], psums[2 + dt][:], AF.Square,
                                     bias=bias_sb[:, 2 + dt:3 + dt])
                nc.tensor.matmul(ssk[:], oh_sb[:, 0:1], sqk[:],
                                 start=(dt == 0), stop=(dt == 1))
            ssr_k = p1sc.tile([1, CHUNK], FP32, tag="ssrk", name=f"ssrk_{c}")
            nc.scalar.activation(ssr_k[:], ssk[:], AF.Copy)
            nc.sync.dma_start(ss_dram[hfc, 1:2, hcs:hcs + CHUNK], ssr_k[:])
            # v: evict bf16, PE-transpose to row-major, one DMA out
            for dt in range(2):
                vst = p1sc.tile([128, CHUNK], BF16, tag="vst",
                                name=f"vst_{c}_{dt}")
                nc.scalar.activation(vst[:], psums[4 + dt][:], AF.Identity,
                                     bias=bias_sb[:, 4 + dt:5 + dt])
                for q in range(4):
                    vtp = p1tp.tile([128, 128], BF16, tag="vtp",
                                    name=f"vtp_{c}_{dt}_{q}")
                    nc.tensor.transpose(vtp[:], vst[:, 128 * q:128 * (q + 1)],
                                        id_sb[:])
                    nc.scalar.activation(
                        vrm[:, q, 128 * dt:128 * (dt + 1)], vtp[:], AF.Copy)
            nc.sync.dma_start(
                v_dram[cs:cs + CHUNK, :].rearrange("(q p) d -> p q d", p=128),
                vrm[:])
            if c == NCHUNK // 2 - 1:
                emit_rms_half(0)
            elif c == NCHUNK - 1:
                emit_rms_half(1)

    # ---------------- P3: normalize k, build qm ----------------
    with tc.tile_pool(name="p3r", bufs=2) as p3r, \
         tc.tile_pool(name="p3ps", bufs=2, space="PSUM") as p3ps, \
         tc.tile_pool(name="p3b", bufs=2) as p3b:
        for c in range(NCHUNK):
            cs = c * CHUNK
            rq_t = p3r.tile([1, CHUNK], FP32, tag="rq")
            rk_t = p3r.tile([1, CHUNK], FP32, tag="rk")
            nc.sync.dma_start(rq_t[:], rms_dram[0:1, cs:cs + CHUNK])
            nc.sync.dma_start(rk_t[:], rms_dram[1:2, cs:cs + CHUNK])
            rb_ps = p3ps.tile([128, CHUNK], FP32, tag="rbps")
            nc.tensor.matmul(rb_ps[:], onesr_sb[:], rq_t[:], start=True, stop=True)
            rqb = p3b.tile([128, CHUNK], BF16, tag="rqb")
            nc.scalar.activation(rqb[:], rb_ps[:], AF.Copy)
            rb_ps2 = p3ps.tile([128, CHUNK], FP32, tag="rbps")
            nc.tensor.matmul(rb_ps2[:], onesr_sb[:], rk_t[:], start=True, stop=True)
            rkb = p3b.tile([128, CHUNK], BF16, tag="rkb")
            nc.scalar.activation(rkb[:], rb_ps2[:], AF.Copy)
            for dt in range(2):
                sl = slice(dt * N + cs, dt * N + cs + CHUNK)
                nc.vector.tensor_tensor(k_sb[:, sl], k_sb[:, sl], rkb[:],
                                        op=AluOp.mult)
            for h in range(HPC):
                qm_ps = p3ps.tile([128, CHUNK], FP32, tag="qmps")
                nc.tensor.matmul(qm_ps[:], pm_sb[:, 128 * h:128 * (h + 1)],
                                 q_raw[:, h * N + cs: h * N + cs + CHUNK],
                                 start=True, stop=True)
                nc.vector.tensor_tensor(qm_sb[:, h * N + cs: h * N + cs + CHUNK],
                                        qm_ps[:], rqb[:], op=AluOp.mult)

    # ------- P4..P7 unified: per group of 4 chunks, scores then apply -------
    # Scores products run on GpSimd so the DVE is free for the previous
    # block's apply chain; trace order interleaves scores(g) and apply(g-1).
    NBLK = 4
    BROWS = N // NBLK          # 2048
    BT = BROWS // 128          # 16
    with tc.tile_pool(name="p4p", bufs=3) as p4p, \
         tc.tile_pool(name="p4ps", bufs=2, space="PSUM") as p4ps, \
         tc.tile_pool(name="p5ps", bufs=2, space="PSUM") as p5ps, \
         tc.tile_pool(name="p5s", bufs=3) as p5s, \
         tc.tile_pool(name="p6v", bufs=2) as p6v, \
         tc.tile_pool(name="p6a", bufs=1) as p6a, \
         tc.tile_pool(name="p6tp", bufs=1, space="PSUM") as p6tp, \
         tc.tile_pool(name="p7ps", bufs=2, space="PSUM") as p7ps, \
         tc.tile_pool(name="p7s", bufs=2) as p7s, \
         tc.tile_pool(name="p6o", bufs=3) as p6o:

        def emit_scores_chunk(c):
            cs = c * CHUNK
            b = cs // L
            w0 = cs - b * L
            for h in range(HPC):
                sc_ps = p4ps.tile([16, CHUNK], FP32, tag="scps",
                                  name=f"scps_{c}_{h}")
                for i, s in enumerate(SEQ_SHIFTS):
                    prod = p4p.tile([128, CHUNK], BF16, tag="prod",
                                    name=f"prod_{c}_{h}_{i}")
                    for (joff, srcl, rl) in _wrap_runs(w0 - s, CHUNK):
                        nc.gpsimd.tensor_tensor(
                            prod[:, joff:joff + rl],
                            qm_sb[:, h * N + cs + joff: h * N + cs + joff + rl],
                            k_sb[:, h * N + b * L + srcl: h * N + b * L + srcl + rl],
                            op=AluOp.mult)
                    nc.tensor.matmul(sc_ps[:], oh_sb[:, 16 * i:16 * (i + 1)],
                                     prod[:], start=(i == 0), stop=(i == NS - 1))
                nc.scalar.activation(exp_sb[0:16, h * N + cs: h * N + cs + CHUNK],
                                     sc_ps[:], AF.Exp)
                for tq in range(4):
                    t = c * 4 + tq
                    tr_ps = p5ps.tile([128, 16], BF16, tag="trps",
                                      name=f"trps_{c}_{h}_{tq}")
                    nc.tensor.transpose(
                        tr_ps[:],
                        exp_sb[0:16, h * N + 128 * t: h * N + 128 * (t + 1)],
                        id_sb[0:16, 0:16])
                    attx = p5s.tile([128, 16], FP32, tag="attx",
                                    name=f"attx_{c}_{h}_{tq}")
                    nc.scalar.activation(attx[:], tr_ps[:], AF.Copy)
                    dsum = p5s.tile([128, 1], FP32, tag="dsum",
                                    name=f"dsum_{c}_{h}_{tq}")
                    nc.vector.tensor_reduce(dsum[:], attx[:],
                                            axis=mybir.AxisListType.X,
                                            op=AluOp.add)
                    rc = p5s.tile([128, 1], FP32, tag="rc",
                                  name=f"rc_{c}_{h}_{tq}")
                    nc.vector.reciprocal(rc[:], dsum[:])
                    nc.vector.tensor_scalar(
                        attn_all[:, (t * HPC + h) * 16: (t * HPC + h) * 16 + 16],
                        attx[:], rc[:], None, op0=AluOp.mult)

        def emit_apply_block(g):
            b = (g * BROWS) // L
            w0 = g * BROWS - b * L
            acc_a = p6a.tile([128, BT, DPC], BF16, tag="acca", name=f"acca_{g}")
            acc_b = p6a.tile([128, BT, DPC], BF16, tag="accb", name=f"accb_{g}")
            for i, s in enumerate(SEQ_SHIFTS):
                vrb = p6v.tile([128, BT, DPC], BF16, tag="vrb",
                               name=f"vrb_{g}_{i}")
                eng = nc.sync if i % 2 == 0 else nc.scalar
                for (joff, srcr, rl) in _wrap_runs(w0 - s, BROWS):
                    ja = joff
                    while ja < joff + rl:
                        t0, p0 = divmod(ja, 128)
                        take = min(joff + rl - ja, 128 - p0)
                        if p0 == 0 and take == 128:
                            nt = (joff + rl - ja) // 128
                            eng.dma_start(
                                vrb[:, t0:t0 + nt, :],
                                v_dram[b * L + srcr + (ja - joff):
                                       b * L + srcr + (ja - joff) + nt * 128, :]
                                .rearrange("(t p) d -> p t d", p=128))
                            ja += nt * 128
                        else:
                            eng.dma_start(
                                vrb[p0:p0 + take, t0, :],
                                v_dram[b * L + srcr + (ja - joff):
                                       b * L + srcr + (ja - joff) + take, :])
                            ja += take
                acc = acc_a if i < 8 else acc_b
                first = i == 0 or i == 8
                for t16 in range(BT):
                    t = g * BT + t16
                    for h in range(HPC):
                        a_col = (t * HPC + h) * 16 + i
                        sl = slice(128 * h, 128 * (h + 1))
                        if first:
                            nc.vector.tensor_scalar(
                                acc[:, t16, sl], vrb[:, t16, sl],
                                attn_all[:, a_col:a_col + 1], None,
                                op0=AluOp.mult)
                        else:
                            nc.vector.scalar_tensor_tensor(
                                acc[:, t16, sl], vrb[:, t16, sl],
                                attn_all[:, a_col:a_col + 1], acc[:, t16, sl],
                                op0=AluOp.mult, op1=AluOp.add)
            for t16 in range(BT):
                t = g * BT + t16
                oa = p6o.tile([128, DPC], BF16, tag="oa", name=f"oa_{g}_{t16}")
                nc.vector.tensor_tensor(oa[:], acc_a[:, t16, :],
                                        acc_b[:, t16, :], op=AluOp.add)
                for dt in range(2):
                    otp = p6tp.tile([128, 128], BF16, tag="otp",
                                    name=f"otp_{g}_{t16}_{dt}")
                    nc.tensor.transpose(otp[:], oa[:, 128 * dt:128 * (dt + 1)],
                                        id_sb[:])
                    nc.scalar.activation(
                        qm_sb[:, dt * N + 128 * t: dt * N + 128 * (t + 1)],
                        otp[:], AF.Copy)
                ost = p7s.tile([128, D], BF16, tag="ost", name=f"ost_{t}")
                for e in range(4):
                    ops = p7ps.tile([128, 512], FP32, tag="ops",
                                    name=f"ops_{t}_{e}")
                    for dt in range(2):
                        nc.tensor.matmul(
                            ops[:],
                            qm_sb[:, dt * N + 128 * t: dt * N + 128 * (t + 1)],
                            wo_sb[:, dt * D + 512 * e: dt * D + 512 * (e + 1)],
                            start=(dt == 0), stop=(dt == 1))
                    nc.scalar.activation(ost[:, 512 * e:512 * (e + 1)], ops[:],
                                         AF.Copy)
                nc.scalar.dma_start(outp[128 * t:128 * (t + 1), :], ost[:])

        for g in range(NBLK):
            for c in range(4 * g, 4 * g + 4):
                emit_scores_chunk(c)
            emit_apply_block(g)


_PROG = None


def _get_program():
    global _PROG
    if _PROG is None:
        _PROG = _build_program()
    return _PROG


def _host_prep(inputs):
    x = np.asarray(inputs['x'], np.float32)
    wq = np.asarray(inputs['wq'], np.float32)
    wk = np.asarray(inputs['wk'], np.float32)
    wv = np.asarray(inputs['wv'], np.float32)
    bq = np.asarray(inputs['bq'], np.float32)
    bk = np.asarray(inputs['bk'], np.float32)
    bv = np.asarray(inputs['bv'], np.float32)
    qnw = np.asarray(inputs['q_norm_w'], np.float32)
    knw = np.asarray(inputs['k_norm_w'], np.float32)
    mix = np.asarray(inputs['score_mix_w'], np.float32)[0]
    wo = np.asarray(inputs['wo'], np.float32)

    xT = np.ascontiguousarray(x.reshape(N, D).T).astype(BF)
    scale = DH ** -0.5

    oh = np.zeros((128, NS * 16), np.float32)
    for i in range(NS):
        oh[:, 16 * i + i] = 1.0
    oh = oh.astype(BF)
    idm = np.eye(128, dtype=np.float32).astype(BF)
    ones_r = np.ones((1, 128), np.float32)

    in_maps = []
    for c in range(NCORES):
        cs = c * DPC
        sl = slice(cs, cs + DPC)
        wTc = np.concatenate([wq[sl].T, wk[sl].T, wv[sl].T], axis=1)  # [2048, 768]
        bias = np.stack([bq[cs:cs + 128], bq[cs + 128:cs + 256],
                         bk[cs:cs + 128], bk[cs + 128:cs + 256],
                         bv[cs:cs + 128], bv[cs + 128:cs + 256]], axis=1)
        pmT = np.zeros((128, HPC * 128), np.float32)
        for h in range(HPC):
            gh = c * HPC + h
            Pm = np.zeros((DH, DH), np.float32)
            for n, ch in enumerate([0] + CH_SHIFTS):
                for dd in range(DH):
                    dp = (dd - ch) % DH
                    Pm[dd, dp] += mix[n] * qnw[gh * DH + dp]
            Pm *= scale * knw[gh * DH:(gh + 1) * DH][:, None]
            pmT[:, 128 * h:128 * (h + 1)] = Pm.T
        woTc = np.ascontiguousarray(wo[:, sl].T)  # [256, 2048]
        in_maps.append({
            "xT": xT,
            "wT": wTc.astype(BF),
            "bias6": np.ascontiguousarray(bias),
            "pmT": pmT.astype(BF),
            "oh": oh,
            "idm": idm,
            "ones_r": ones_r,
            "woT": woTc.astype(BF),
        })
    return in_maps


LAST_RESULT = None


def kernel(**inputs):
    global LAST_RESULT
    import os
    in_maps = _host_prep(inputs)
    nc = _get_program()
    trace = bool(os.environ.get("CRA_TRACE"))
    res = run_bass_kernel_spmd(nc, in_maps, list(range(NCORES)), trace=trace)
    LAST_RESULT = res
    acc = np.zeros((N, D), np.float32)
    for r in res.results:
        acc += np.asarray(r["outp"], np.float32)
    acc += np.asarray(inputs['bo'], np.float32)
    return acc.reshape(B, L, D)



# revision 6
# speedup vs baseline: 1.1722x; 1.1722x over previous
"""Trainium2 Bass kernel for nn_CliffordRollingAttention.

Strategy (head-parallel over 8 cores, 2 heads/core), v2 fused pipeline:
  - Host pre-transposes x -> xT [D, B*L] bf16, slices/folds weights per core.
  - On-device per core, per batch half:
      A: QKV projections on PE in transposed layout [d, l] (2 rotating PSUM
         banks, m-outer over 256-l subchunks), bf16 with fp32 PSUM.
         Per-row sumsq partials for RMS via ACT Square + PE ones-reduce.
      B: 64KB AllReduce of q/k sumsq partials, rsqrt -> bf16 rms rows.
      C (per 512-l chunk, fused): rms rows broadcast via gpsimd
         partition_broadcast; k-norm + qm build (PE matmul w/ host-folded
         mix matrix, DVE multiply); scores = DVE products qm*k[:, l-s]
         + PE one-hot reduce into [16, l] PSUM; ACT exp; denominator via
         PE ones-reduce + DVE reciprocal; exp rows flattened to partition 0
         (SBUF DMA) then gpsimd partition_broadcast to [128, l]; apply
         entirely in transposed layout: acc += eb_i * v^T[:, l-s] on DVE
         (v^T kept resident in SBUF, never transposed or respilled);
         normalize once by broadcast reciprocal; output projection directly
         from the transposed accumulator (PE), row-major PSUM -> bf16 out.
  - Emission interleaves batch-1 projections with batch-0 fused chunks so
    PE stays busy while DVE/GpSimd work, and vice versa.
  - Host sums the 8 partial outputs in fp32 and adds the output bias.
"""

import numpy as np
import ml_dtypes

import concourse.bass as bass
import concourse.bacc as bacc
import concourse.mybir as mybir
import concourse.tile as tile
from concourse import library_config
from concourse.bass_utils import run_bass_kernel_spmd

BF = ml_dtypes.bfloat16
FP32 = mybir.dt.float32
BF16 = mybir.dt.bfloat16

B, L, D = 2, 4096, 2048
H, DH = 16, 128
NCORES = 8
HPC = H // NCORES          # heads per core = 2
DPC = HPC * DH             # channels per core = 256
N = B * L                  # 8192 rows
EPS = 1e-6
SEQ_SHIFTS = [0, 1, -1, 3, -3, 9, -9, 26, -26, 78, -78, 232, -232, 689, -689, 2048]
CH_SHIFTS = [1, 2, 4, 8]
NS = len(SEQ_SHIFTS)       # 16
CHUNK = 512
NCHUNK = N // CHUNK        # 16
SUB = 256                  # P1 subchunk
AluOp = mybir.AluOpType
AF = mybir.ActivationFunctionType


def _wrap_runs(start, length):
    """Split output positions j in [0,length) whose source row is
    b*L + ((start + j) mod L) into maximal contiguous source runs.
    Returns list of (j_offset, src_local_start, run_len)."""
    runs = []
    j = 0
    while j < length:
        src = (start + j) % L
        run = min(length - j, L - src)
        runs.append((j, src, run))
        j += run
    return runs


def _build_program():
    nc = bacc.Bacc(num_devices=NCORES)

    handles = {
        "xT": nc.declare_dram_parameter("xT", [D, N], BF16, isOutput=False),
        "wT": nc.declare_dram_parameter("wT", [D, 6 * 128], BF16, isOutput=False),
        "bias6": nc.declare_dram_parameter("bias6", [128, 6], FP32, isOutput=False),
        "pmT": nc.declare_dram_parameter("pmT", [128, HPC * 128], BF16, isOutput=False),
        "oh": nc.declare_dram_parameter("oh", [128, NS * 16], BF16, isOutput=False),
        "ones16": nc.declare_dram_parameter("ones16", [16, 1], BF16, isOutput=False),
        "woT": nc.declare_dram_parameter("woT", [DPC, D], BF16, isOutput=False),
        "outp": nc.declare_dram_parameter("outp", [N, D], BF16, isOutput=True),
    }

    import contextlib
    with tile.TileContext(nc) as tc:
        with contextlib.ExitStack() as ctx:
            _emit_inner(ctx, tc, handles)
    nc.compile()
    return nc


def _emit_inner(ctx, tc, handles):
    nc = tc.nc
    xT = handles["xT"][:]
    wT = handles["wT"][:]
    bias6 = handles["bias6"][:]
    pmT_d = handles["pmT"][:]
    oh_d = handles["oh"][:]
    ones16_d = handles["ones16"][:]
    woT_d = handles["woT"][:]
    outp = handles["outp"][:]

    nc.gpsimd.load_library(library_config.proxy)

    # ---------------- persistent pools ----------------
    const = ctx.enter_context(tc.tile_pool(name="const", bufs=1))
    big = ctx.enter_context(tc.tile_pool(name="big", bufs=1))
    dram = ctx.enter_context(tc.tile_pool(name="dram", bufs=1, space="DRAM"))

    w_sb = const.tile([128, 16 * 768], BF16)        # 24KB
    bias_sb = const.tile([128, 6], FP32)
    pm_sb = const.tile([128, HPC * 128], BF16)
    oh_sb = const.tile([128, NS * 16], BF16)
    ones16_sb = const.tile([16, 1], BF16)
    eps_sb = const.tile([128, 1], FP32)
    wo_sb = const.tile([128, HPC * D], BF16)        # 8KB [128, dt*2048 + e]

    nc.sync.dma_start(w_sb[:].rearrange("p (k j) -> p k j", k=16),
                      wT.rearrange("(k p) j -> p k j", p=128))
    nc.sync.dma_start(bias_sb[:], bias6)
    nc.sync.dma_start(pm_sb[:], pmT_d)
    nc.sync.dma_start(oh_sb[:], oh_d)
    nc.sync.dma_start(ones16_sb[:], ones16_d)
    nc.gpsimd.memset(eps_sb[:], EPS)
    nc.sync.dma_start(wo_sb[:].rearrange("p (dt e) -> p dt e", dt=2),
                      woT_d.rearrange("(dt p) e -> p dt e", p=128))

    q_raw = big.tile([128, HPC * N], BF16)               # 32KB [p, h*N + l]
    k_sb = big.tile([128, HPC * N], BF16)                # 32KB
    v_sb = big.tile([128, HPC * N], BF16)                # 32KB

    q_view = q_raw[:].rearrange("p (h l) -> p h l", h=2)
    k_view = k_sb[:].rearrange("p (h l) -> p h l", h=2)
    v_view = v_sb[:].rearrange("p (h l) -> p h l", h=2)

    ss_dram = dram.tile([2, 2, N // 2], FP32)   # [half, q/k, l-in-half]
    ss_out = dram.tile([2, 2, N // 2], FP32)
    rms_dram = dram.tile([2, N], BF16)

    # ---------------- working pools ----------------
    p1x = ctx.enter_context(tc.tile_pool(name="p1x", bufs=2))
    sqp = ctx.enter_context(tc.tile_pool(name="sqp", bufs=2))
    fp = ctx.enter_context(tc.tile_pool(name="fp", bufs=2))
    fp1 = ctx.enter_context(tc.tile_pool(name="fp1", bufs=1))
    p1ps = ctx.enter_context(tc.tile_pool(name="p1ps", bufs=1, space="PSUM"))
    miscps = ctx.enter_context(tc.tile_pool(name="miscps", bufs=2, space="PSUM"))
    scps = ctx.enter_context(tc.tile_pool(name="scps", bufs=1, space="PSUM"))
    p7ps = ctx.enter_context(tc.tile_pool(name="p7ps", bufs=2, space="PSUM"))

    # ---------------- stage B: AllReduce + rsqrt ----------------
    def emit_rms_half(hf):
        HN = N // 2
        nc.gpsimd.collective_compute(
            "AllReduce", AluOp.add,
            replica_groups=[list(range(NCORES))],
            ins=[ss_dram[hf].opt()],
            outs=[ss_out[hf].opt()],
        )
        with tc.tile_pool(name=f"p2_{hf}", bufs=1) as p2:
            col = p2.tile([128, 64], FP32, name=f"col_{hf}")
            srt = p2.tile([128, 64], FP32, name=f"srt_{hf}")
            rinv = p2.tile([128, 64], BF16, name=f"rinv_{hf}")
            for r in range(2):
                nc.sync.dma_start(
                    col[:, r * 32:(r + 1) * 32],
                    ss_out[hf, r, :].rearrange("(t p) -> p t", p=128))
            nc.scalar.activation(srt[:], col[:], AF.Sqrt, bias=eps_sb[:],
                                 scale=1.0 / D)
            with nc.allow_low_precision(reason="bf16 rms factors, tol 2e-2"):
                nc.vector.reciprocal(rinv[:], srt[:])
            for r in range(2):
                nc.sync.dma_start(
                    rms_dram[r, hf * HN:(hf + 1) * HN]
                    .rearrange("(t p) -> p t", p=128),
                    rinv[:, r * 32:(r + 1) * 32])

    # ---------------- stage A: projections (one 256-l subchunk) ------------
    def emit_p1_sub(si):
        cs = si * SUB
        hfc, hcs = divmod(cs, N // 2)
        xt = p1x.tile([128, 16, SUB], BF16, tag="xt", name=f"xt_{si}")
        nc.sync.dma_start(xt[:],
                          xT[:, cs:cs + SUB].rearrange("(k p) l -> p k l", p=128))
        sqs = {}
        # m order: q0 q1 k0 k1 v0 v1
        for m in range(6):
            ps = p1ps.tile([128, SUB], FP32, tag=f"ps{m % 2}", name=f"ps{m}_{si}")
            for k in range(16):
                nc.tensor.matmul(
                    ps[:],
                    w_sb[:, k * 768 + 128 * m: k * 768 + 128 * (m + 1)],
                    xt[:, k, :],
                    start=(k == 0), stop=(k == 15),
                )
            kind, dt = divmod(m, 2)
            if kind == 0:    # q
                nc.scalar.activation(q_raw[:, dt * N + cs: dt * N + cs + SUB],
                                     ps[:], AF.Identity, bias=bias_sb[:, dt:dt + 1])
                sq = sqp.tile([128, SUB], BF16, tag=f"sq{dt}", name=f"sq{dt}_{si}")
                nc.scalar.activation(sq[:], ps[:], AF.Square,
                                     bias=bias_sb[:, dt:dt + 1])
                sqs[f"q{dt}"] = sq
            elif kind == 1:  # k
                nc.scalar.activation(k_sb[:, dt * N + cs: dt * N + cs + SUB],
                                     ps[:], AF.Identity, bias=bias_sb[:, 2 + dt:3 + dt])
                sq = sqp.tile([128, SUB], BF16, tag=f"sqk{dt}", name=f"sqk{dt}_{si}")
                nc.scalar.activation(sq[:], ps[:], AF.Square,
                                     bias=bias_sb[:, 2 + dt:3 + dt])
                sqs[f"k{dt}"] = sq
            else:            # v
                nc.scalar.activation(v_sb[:, dt * N + cs: dt * N + cs + SUB],
                                     ps[:], AF.Identity, bias=bias_sb[:, 4 + dt:5 + dt])
        # sumsq reduce via PE (ones column of oh), evict, ship to DRAM
        for kind, key in ((0, "q"), (1, "k")):
            ssq = miscps.tile([1, 512], FP32, tag="misc", name=f"ssq{key}_{si}")
            for dt in range(2):
                nc.tensor.matmul(ssq[:, 0:SUB], oh_sb[:, 0:1], sqs[f"{key}{dt}"][:],
                                 start=(dt == 0), stop=(dt == 1))
            ssr = sqp.tile([1, SUB], FP32, tag=f"ssr{key}", name=f"ssr{key}_{si}")
            nc.scalar.activation(ssr[:], ssq[:, 0:SUB], AF.Copy)
            nc.sync.dma_start(ss_dram[hfc, kind:kind + 1, hcs:hcs + SUB], ssr[:])

    # ---------------- stage C: fused chunk ----------------
    def emit_fused(c):
        cs = c * CHUNK
        b = cs // L
        w0 = cs - b * L
        bL = b * L
        # rms rows -> broadcast
        rq = fp.tile([1, CHUNK], BF16, tag="rq", name=f"rq_{c}")
        rk = fp.tile([1, CHUNK], BF16, tag="rk", name=f"rk_{c}")
        nc.sync.dma_start(rq[:], rms_dram[0:1, cs:cs + CHUNK])
        nc.sync.dma_start(rk[:], rms_dram[1:2, cs:cs + CHUNK])
        rqb = fp.tile([128, CHUNK], BF16, tag="rqb", name=f"rqb_{c}")
        rkb = fp.tile([128, CHUNK], BF16, tag="rkb", name=f"rkb_{c}")
        nc.gpsimd.partition_broadcast(rqb[:], rq[:])
        nc.gpsimd.partition_broadcast(rkb[:], rk[:])
        # k-norm
        for dt in range(2):
            nc.vector.tensor_tensor(k_view[:, dt, cs:cs + CHUNK],
                                    k_view[:, dt, cs:cs + CHUNK], rkb[:],
                                    op=AluOp.mult)
        # qm
        qmc = fp.tile([128, 2, CHUNK], BF16, tag="qot", name=f"qm_{c}")
        for h in range(HPC):
            qm_ps = miscps.tile([128, CHUNK], FP32, tag="misc", name=f"qmps_{c}_{h}")
            nc.tensor.matmul(qm_ps[:], pm_sb[:, 128 * h:128 * (h + 1)],
                             q_view[:, h, cs:cs + CHUNK], start=True, stop=True)
            nc.vector.tensor_tensor(qmc[:, h, :], qm_ps[:], rqb[:], op=AluOp.mult)
        # scores
        sc = scps.tile([16, 2, CHUNK], FP32, tag="sc", name=f"sc_{c}")
        for i, s in enumerate(SEQ_SHIFTS):
            pr = fp.tile([128, 2, CHUNK], BF16, tag="prod", name=f"pr_{c}_{i}")
            for (joff, srcl, rl) in _wrap_runs(w0 - s, CHUNK):
                nc.vector.tensor_tensor(
                    pr[:, :, joff:joff + rl],
                    qmc[:, :, joff:joff + rl],
                    k_view[:, :, bL + srcl: bL + srcl + rl],
                    op=AluOp.mult)
            for h in range(HPC):
                nc.tensor.matmul(sc[:, h, :], oh_sb[:, 16 * i:16 * (i + 1)],
                                 pr[:, h, :], start=(i == 0), stop=(i == NS - 1))
        # exp + den + recip
        ec = fp.tile([16, 2, CHUNK], BF16, tag="expc", name=f"ec_{c}")
        rr = fp.tile([1, 2, CHUNK], BF16, tag="rrow", name=f"rr_{c}")
        for h in range(HPC):
            nc.scalar.activation(ec[:, h, :], sc[:, h, :], AF.Exp)
        for h in range(HPC):
            dn = miscps.tile([1, CHUNK], FP32, tag="misc", name=f"dn_{c}_{h}")
            nc.tensor.matmul(dn[:], ones16_sb[:, 0:1], ec[:, h, :],
                             start=True, stop=True)
            with nc.allow_low_precision(reason="bf16 softmax recip, tol 2e-2"):
                nc.vector.reciprocal(rr[:, h, :], dn[:])
        rbt = fp1.tile([128, 2, CHUNK], BF16, tag="rb", name=f"rb_{c}")
        nc.gpsimd.partition_broadcast(rbt[:], rr[:])
        # apply, transposed: acc += bcast(exp_i) * v^T[:, l - s_i]
        acc_a = fp.tile([128, 2, CHUNK], BF16, tag="acca", name=f"acca_{c}")
        acc_b = fp.tile([128, 2, CHUNK], BF16, tag="accb", name=f"accb_{c}")
        for w in range(8):
            ef = fp.tile([1, 2, 2, CHUNK], BF16, tag="eflat", name=f"ef_{c}_{w}")
            nc.sync.dma_start(ef[:], ec[2 * w:2 * w + 2, :, :])
            for j in range(2):
                i = 2 * w + j
                s = SEQ_SHIFTS[i]
                eb = fp.tile([128, 2, CHUNK], BF16, tag="eb", name=f"eb_{c}_{i}")
                nc.gpsimd.partition_broadcast(eb[:], ef[0:1, j, :, :])
                acc = acc_a if i < 8 else acc_b
                first = i in (0, 8)
                tgt = acc if first else fp.tile([128, 2, CHUNK], BF16,
                                                 tag="prod", name=f"ap_{c}_{i}")
                for (joff, srcl, rl) in _wrap_runs(w0 - s, CHUNK):
                    nc.vector.tensor_tensor(
                        tgt[:, :, joff:joff + rl],
                        eb[:, :, joff:joff + rl],
                        v_view[:, :, bL + srcl: bL + srcl + rl],
                        op=AluOp.mult)
                if not first:
                    nc.vector.tensor_tensor(acc[:], acc[:], tgt[:], op=AluOp.add)
        outT = fp.tile([128, 2, CHUNK], BF16, tag="qot", name=f"outT_{c}")
        nc.vector.tensor_tensor(acc_a[:], acc_a[:], acc_b[:], op=AluOp.add)
        nc.vector.tensor_tensor(outT[:], acc_a[:], rbt[:], op=AluOp.mult)
        # output projection
        for t4 in range(4):
            t = c * 4 + t4
            for half in range(2):
                ost = fp.tile([128, D // 2], BF16, tag="ost", name=f"ost_{t}_{half}")
                for e2 in range(2):
                    e = half * 2 + e2
                    ops = p7ps.tile([128, 512], FP32, tag="p7", name=f"ops_{t}_{e}")
                    for dt in range(2):
                        nc.tensor.matmul(
                            ops[:],
                            outT[:, dt, 128 * t4:128 * (t4 + 1)],
                            wo_sb[:, dt * D + 512 * e: dt * D + 512 * (e + 1)],
                            start=(dt == 0), stop=(dt == 1))
                    nc.scalar.activation(ost[:, 512 * e2:512 * (e2 + 1)], ops[:],
                                         AF.Copy)
                nc.scalar.dma_start(
                    outp[128 * t:128 * (t + 1), half * (D // 2):(half + 1) * (D // 2)],
                    ost[:])

    # ---------------- schedule ----------------
    SPC = CHUNK // SUB  # subchunks per chunk = 2
    # batch 0 projections
    for c in range(8):
        for s in range(SPC):
            emit_p1_sub(c * SPC + s)
    emit_rms_half(0)
    # fused batch 0 interleaved with batch 1 projections
    for idx in range(8):
        emit_fused(idx)
        for s in range(SPC):
            emit_p1_sub((8 + idx) * SPC + s)
    emit_rms_half(1)
    for idx in range(8, 16):
        emit_fused(idx)


_PROG = None


def _get_program():
    global _PROG
    if _PROG is None:
        _PROG = _build_program()
    return _PROG


def _host_prep(inputs):
    wq = np.asarray(inputs['wq'], np.float32)
    wk = np.asarray(inputs['wk'], np.float32)
    wv = np.asarray(inputs['wv'], np.float32)
    bq = np.asarray(inputs['bq'], np.float32)
    bk = np.asarray(inputs['bk'], np.float32)
    bv = np.asarray(inputs['bv'], np.float32)
    qnw = np.asarray(inputs['q_norm_w'], np.float32)
    knw = np.asarray(inputs['k_norm_w'], np.float32)
    mix = np.asarray(inputs['score_mix_w'], np.float32)[0]
    wo = np.asarray(inputs['wo'], np.float32)

    x = np.asarray(inputs['x'], np.float32)
    xT = np.ascontiguousarray(x.reshape(N, D).T).astype(BF)
    scale = DH ** -0.5

    oh = np.zeros((128, NS * 16), np.float32)
    for i in range(NS):
        oh[:, 16 * i + i] = 1.0
    oh = oh.astype(BF)
    ones16 = np.ones((16, 1), np.float32).astype(BF)

    in_maps = []
    for c in range(NCORES):
        cs = c * DPC
        sl = slice(cs, cs + DPC)
        wTc = np.concatenate([wq[sl].T, wk[sl].T, wv[sl].T], axis=1)  # [2048, 768]
        bias = np.stack([bq[cs:cs + 128], bq[cs + 128:cs + 256],
                         bk[cs:cs + 128], bk[cs + 128:cs + 256],
                         bv[cs:cs + 128], bv[cs + 128:cs + 256]], axis=1)
        pmT = np.zeros((128, HPC * 128), np.float32)
        for h in range(HPC):
            gh = c * HPC + h
            Pm = np.zeros((DH, DH), np.float32)
            for n, ch in enumerate([0] + CH_SHIFTS):
                for dd in range(DH):
                    dp = (dd - ch) % DH
                    Pm[dd, dp] += mix[n] * qnw[gh * DH + dp]
            Pm *= scale * knw[gh * DH:(gh + 1) * DH][:, None]
            pmT[:, 128 * h:128 * (h + 1)] = Pm.T
        woTc = np.ascontiguousarray(wo[:, sl].T)  # [256, 2048]
        in_maps.append({
            "xT": xT,
            "wT": wTc.astype(BF),
            "bias6": np.ascontiguousarray(bias),
            "pmT": pmT.astype(BF),
            "oh": oh,
            "ones16": ones16,
            "woT": woTc.astype(BF),
        })
    return in_maps


LAST_RESULT = None


def kernel(**inputs):
    global LAST_RESULT
    import os
    in_maps = _host_prep(inputs)
    nc = _get_program()
    trace = bool(os.environ.get("CRA_TRACE"))
    res = run_bass_kernel_spmd(nc, in_maps, list(range(NCORES)), trace=trace)
    LAST_RESULT = res
    acc = np.zeros((N, D), np.float32)
    for r in res.results:
        acc += np.asarray(r["outp"], np.float32)
    acc += np.asarray(inputs['bo'], np.float32)
    return acc.reshape(B, L, D)


# revision 7
# speedup vs baseline: 1.1745x; 1.0020x over previous
"""Trainium2 Bass kernel for nn_CliffordRollingAttention.

Strategy (head-parallel over 8 cores, 2 heads/core), v2 fused pipeline:
  - Host pre-transposes x -> xT [D, B*L] bf16, slices/folds weights per core.
  - On-device per core, per batch half:
      A: QKV projections on PE in transposed layout [d, l] (2 rotating PSUM
         banks, m-outer over 256-l subchunks), bf16 with fp32 PSUM.
         Per-row sumsq partials for RMS via ACT Square + PE ones-reduce.
      B: 64KB AllReduce of q/k sumsq partials, rsqrt -> bf16 rms rows.
      C (per 512-l chunk, fused): rms rows broadcast via gpsimd
         partition_broadcast; k-norm + qm build (PE matmul w/ host-folded
         mix matrix, DVE multiply); scores = DVE products qm*k[:, l-s]
         + PE one-hot reduce into [16, l] PSUM; ACT exp; denominator via
         PE ones-reduce + DVE reciprocal; exp rows flattened to partition 0
         (SBUF DMA) then gpsimd partition_broadcast to [128, l]; apply
         entirely in transposed layout: acc += eb_i * v^T[:, l-s] on DVE
         (v^T kept resident in SBUF, never transposed or respilled);
         normalize once by broadcast reciprocal; output projection directly
         from the transposed accumulator (PE), row-major PSUM -> bf16 out.
  - Emission interleaves batch-1 projections with batch-0 fused chunks so
    PE stays busy while DVE/GpSimd work, and vice versa.
  - Host sums the 8 partial outputs in fp32 and adds the output bias.
"""

import numpy as np
import ml_dtypes

import concourse.bass as bass
import concourse.bacc as bacc
import concourse.mybir as mybir
import concourse.tile as tile
from concourse import library_config
from concourse.bass_utils import run_bass_kernel_spmd

BF = ml_dtypes.bfloat16
FP32 = mybir.dt.float32
BF16 = mybir.dt.bfloat16

B, L, D = 2, 4096, 2048
H, DH = 16, 128
NCORES = 8
HPC = H // NCORES          # heads per core = 2
DPC = HPC * DH             # channels per core = 256
N = B * L                  # 8192 rows
EPS = 1e-6
SEQ_SHIFTS = [0, 1, -1, 3, -3, 9, -9, 26, -26, 78, -78, 232, -232, 689, -689, 2048]
CH_SHIFTS = [1, 2, 4, 8]
NS = len(SEQ_SHIFTS)       # 16
CHUNK = 512
NCHUNK = N // CHUNK        # 16
SUB = 256                  # P1 subchunk
AluOp = mybir.AluOpType
AF = mybir.ActivationFunctionType


def _wrap_runs(start, length):
    """Split output positions j in [0,length) whose source row is
    b*L + ((start + j) mod L) into maximal contiguous source runs.
    Returns list of (j_offset, src_local_start, run_len)."""
    runs = []
    j = 0
    while j < length:
        src = (start + j) % L
        run = min(length - j, L - src)
        runs.append((j, src, run))
        j += run
    return runs


def _build_program():
    nc = bacc.Bacc(num_devices=NCORES)

    handles = {
        "xT": nc.declare_dram_parameter("xT", [D, N], BF16, isOutput=False),
        "wT": nc.declare_dram_parameter("wT", [D, 6 * 128], BF16, isOutput=False),
        "bias6": nc.declare_dram_parameter("bias6", [128, 6], FP32, isOutput=False),
        "pmT": nc.declare_dram_parameter("pmT", [128, HPC * 128], BF16, isOutput=False),
        "oh": nc.declare_dram_parameter("oh", [128, NS * 16], BF16, isOutput=False),
        "ones16": nc.declare_dram_parameter("ones16", [16, 1], BF16, isOutput=False),
        "woT": nc.declare_dram_parameter("woT", [DPC, D], BF16, isOutput=False),
        "outp": nc.declare_dram_parameter("outp", [N, D], BF16, isOutput=True),
    }

    import contextlib
    with tile.TileContext(nc) as tc:
        with contextlib.ExitStack() as ctx:
            _emit_inner(ctx, tc, handles)
    nc.compile()
    return nc


def _emit_inner(ctx, tc, handles):
    nc = tc.nc
    xT = handles["xT"][:]
    wT = handles["wT"][:]
    bias6 = handles["bias6"][:]
    pmT_d = handles["pmT"][:]
    oh_d = handles["oh"][:]
    ones16_d = handles["ones16"][:]
    woT_d = handles["woT"][:]
    outp = handles["outp"][:]

    nc.gpsimd.load_library(library_config.proxy)

    # ---------------- persistent pools ----------------
    const = ctx.enter_context(tc.tile_pool(name="const", bufs=1))
    big = ctx.enter_context(tc.tile_pool(name="big", bufs=1))
    dram = ctx.enter_context(tc.tile_pool(name="dram", bufs=1, space="DRAM"))

    w_sb = const.tile([128, 16 * 768], BF16)        # 24KB
    bias_sb = const.tile([128, 6], FP32)
    pm_sb = const.tile([128, HPC * 128], BF16)
    oh_sb = const.tile([128, NS * 16], BF16)
    ones16_sb = const.tile([16, 1], BF16)
    eps_sb = const.tile([128, 1], FP32)
    wo_sb = const.tile([128, HPC * D], BF16)        # 8KB [128, dt*2048 + e]

    nc.sync.dma_start(w_sb[:].rearrange("p (k j) -> p k j", k=16),
                      wT.rearrange("(k p) j -> p k j", p=128))
    nc.sync.dma_start(bias_sb[:], bias6)
    nc.sync.dma_start(pm_sb[:], pmT_d)
    nc.sync.dma_start(oh_sb[:], oh_d)
    nc.sync.dma_start(ones16_sb[:], ones16_d)
    nc.gpsimd.memset(eps_sb[:], EPS)
    nc.sync.dma_start(wo_sb[:].rearrange("p (dt e) -> p dt e", dt=2),
                      woT_d.rearrange("(dt p) e -> p dt e", p=128))

    q_raw = big.tile([128, HPC * N], BF16)               # 32KB [p, h*N + l]
    k_sb = big.tile([128, HPC * N], BF16)                # 32KB
    v_sb = big.tile([128, HPC * N], BF16)                # 32KB

    q_view = q_raw[:].rearrange("p (h l) -> p h l", h=2)
    k_view = k_sb[:].rearrange("p (h l) -> p h l", h=2)
    v_view = v_sb[:].rearrange("p (h l) -> p h l", h=2)

    ss_dram = dram.tile([2, 2, N // 2], FP32)   # [half, q/k, l-in-half]
    ss_out = dram.tile([2, 2, N // 2], FP32)
    rms_dram = dram.tile([2, N], BF16)

    # ---------------- working pools ----------------
    p1x = ctx.enter_context(tc.tile_pool(name="p1x", bufs=2))
    sqp = ctx.enter_context(tc.tile_pool(name="sqp", bufs=2))
    fp = ctx.enter_context(tc.tile_pool(name="fp", bufs=2))
    fp1 = ctx.enter_context(tc.tile_pool(name="fp1", bufs=1))
    p1ps = ctx.enter_context(tc.tile_pool(name="p1ps", bufs=1, space="PSUM"))
    miscps = ctx.enter_context(tc.tile_pool(name="miscps", bufs=2, space="PSUM"))
    scps = ctx.enter_context(tc.tile_pool(name="scps", bufs=1, space="PSUM"))
    p7ps = ctx.enter_context(tc.tile_pool(name="p7ps", bufs=2, space="PSUM"))

    # ---------------- stage B: AllReduce + rsqrt ----------------
    def emit_rms_half(hf):
        HN = N // 2
        nc.gpsimd.collective_compute(
            "AllReduce", AluOp.add,
            replica_groups=[list(range(NCORES))],
            ins=[ss_dram[hf].opt()],
            outs=[ss_out[hf].opt()],
        )
        with tc.tile_pool(name=f"p2_{hf}", bufs=1) as p2:
            col = p2.tile([128, 64], FP32, name=f"col_{hf}")
            srt = p2.tile([128, 64], FP32, name=f"srt_{hf}")
            rinv = p2.tile([128, 64], BF16, name=f"rinv_{hf}")
            for r in range(2):
                nc.sync.dma_start(
                    col[:, r * 32:(r + 1) * 32],
                    ss_out[hf, r, :].rearrange("(t p) -> p t", p=128))
            nc.scalar.activation(srt[:], col[:], AF.Sqrt, bias=eps_sb[:],
                                 scale=1.0 / D)
            with nc.allow_low_precision(reason="bf16 rms factors, tol 2e-2"):
                nc.vector.reciprocal(rinv[:], srt[:])
            for r in range(2):
                nc.sync.dma_start(
                    rms_dram[r, hf * HN:(hf + 1) * HN]
                    .rearrange("(t p) -> p t", p=128),
                    rinv[:, r * 32:(r + 1) * 32])

    # ---------------- stage A: projections (one 256-l subchunk) ------------
    def emit_p1_sub(si):
        cs = si * SUB
        hfc, hcs = divmod(cs, N // 2)
        xt = p1x.tile([128, 16, SUB], BF16, tag="xt", name=f"xt_{si}")
        nc.sync.dma_start(xt[:],
                          xT[:, cs:cs + SUB].rearrange("(k p) l -> p k l", p=128))
        sqs = {}
        # m order: q0 q1 k0 k1 v0 v1
        for m in range(6):
            ps = p1ps.tile([128, SUB], FP32, tag=f"ps{m % 2}", name=f"ps{m}_{si}")
            for k in range(16):
                nc.tensor.matmul(
                    ps[:],
                    w_sb[:, k * 768 + 128 * m: k * 768 + 128 * (m + 1)],
                    xt[:, k, :],
                    start=(k == 0), stop=(k == 15),
                )
            kind, dt = divmod(m, 2)
            if kind == 0:    # q
                nc.scalar.activation(q_raw[:, dt * N + cs: dt * N + cs + SUB],
                                     ps[:], AF.Identity, bias=bias_sb[:, dt:dt + 1])
                sq = sqp.tile([128, SUB], BF16, tag=f"sq{dt}", name=f"sq{dt}_{si}")
                nc.scalar.activation(sq[:], ps[:], AF.Square,
                                     bias=bias_sb[:, dt:dt + 1])
                sqs[f"q{dt}"] = sq
            elif kind == 1:  # k
                nc.scalar.activation(k_sb[:, dt * N + cs: dt * N + cs + SUB],
                                     ps[:], AF.Identity, bias=bias_sb[:, 2 + dt:3 + dt])
                sq = sqp.tile([128, SUB], BF16, tag=f"sqk{dt}", name=f"sqk{dt}_{si}")
                nc.scalar.activation(sq[:], ps[:], AF.Square,
                                     bias=bias_sb[:, 2 + dt:3 + dt])
                sqs[f"k{dt}"] = sq
            else:            # v
                nc.scalar.activation(v_sb[:, dt * N + cs: dt * N + cs + SUB],
                                     ps[:], AF.Identity, bias=bias_sb[:, 4 + dt:5 + dt])
        # sumsq reduce via PE (ones column of oh), evict, ship to DRAM
        for kind, key in ((0, "q"), (1, "k")):
            ssq = miscps.tile([1, 512], FP32, tag="misc", name=f"ssq{key}_{si}")
            for dt in range(2):
                nc.tensor.matmul(ssq[:, 0:SUB], oh_sb[:, 0:1], sqs[f"{key}{dt}"][:],
                                 start=(dt == 0), stop=(dt == 1))
            ssr = sqp.tile([1, SUB], FP32, tag=f"ssr{key}", name=f"ssr{key}_{si}")
            nc.scalar.activation(ssr[:], ssq[:, 0:SUB], AF.Copy)
            nc.sync.dma_start(ss_dram[hfc, kind:kind + 1, hcs:hcs + SUB], ssr[:])

    # ---------------- stage C: fused chunk ----------------
    def emit_fused(c):
        cs = c * CHUNK
        b = cs // L
        w0 = cs - b * L
        bL = b * L
        # rms rows -> broadcast
        rq = fp.tile([1, CHUNK], BF16, tag="rq", name=f"rq_{c}")
        rk = fp.tile([1, CHUNK], BF16, tag="rk", name=f"rk_{c}")
        nc.sync.dma_start(rq[:], rms_dram[0:1, cs:cs + CHUNK])
        nc.sync.dma_start(rk[:], rms_dram[1:2, cs:cs + CHUNK])
        rqb = fp.tile([128, CHUNK], BF16, tag="rqb", name=f"rqb_{c}")
        rkb = fp.tile([128, CHUNK], BF16, tag="rkb", name=f"rkb_{c}")
        nc.gpsimd.partition_broadcast(rqb[:], rq[:])
        nc.gpsimd.partition_broadcast(rkb[:], rk[:])
        # k-norm
        for dt in range(2):
            nc.vector.tensor_tensor(k_view[:, dt, cs:cs + CHUNK],
                                    k_view[:, dt, cs:cs + CHUNK], rkb[:],
                                    op=AluOp.mult)
        # qm
        qmc = fp.tile([128, 2, CHUNK], BF16, tag="qot", name=f"qm_{c}")
        for h in range(HPC):
            qm_ps = miscps.tile([128, CHUNK], FP32, tag="misc", name=f"qmps_{c}_{h}")
            nc.tensor.matmul(qm_ps[:], pm_sb[:, 128 * h:128 * (h + 1)],
                             q_view[:, h, cs:cs + CHUNK], start=True, stop=True)
            nc.vector.tensor_tensor(qmc[:, h, :], qm_ps[:], rqb[:], op=AluOp.mult)
        # scores
        sc = scps.tile([16, 2, CHUNK], FP32, tag="sc", name=f"sc_{c}")
        for i, s in enumerate(SEQ_SHIFTS):
            pr = fp.tile([128, 2, CHUNK], BF16, tag="prod", name=f"pr_{c}_{i}")
            for (joff, srcl, rl) in _wrap_runs(w0 - s, CHUNK):
                for h in range(HPC):
                    nc.vector.tensor_tensor(
                        pr[:, h, joff:joff + rl],
                        qmc[:, h, joff:joff + rl],
                        k_view[:, h, bL + srcl: bL + srcl + rl],
                        op=AluOp.mult)
            for h in range(HPC):
                nc.tensor.matmul(sc[:, h, :], oh_sb[:, 16 * i:16 * (i + 1)],
                                 pr[:, h, :], start=(i == 0), stop=(i == NS - 1))
        # exp + den + recip
        ec = fp.tile([16, 2, CHUNK], BF16, tag="expc", name=f"ec_{c}")
        dflat = fp.tile([1, 2, CHUNK], BF16, tag="rrow", name=f"dflat_{c}")
        for h in range(HPC):
            nc.scalar.activation(ec[:, h, :], sc[:, h, :], AF.Exp)
        for h in range(HPC):
            dn = miscps.tile([1, CHUNK], FP32, tag="misc", name=f"dn_{c}_{h}")
            nc.tensor.matmul(dn[:], ones16_sb[:, 0:1], ec[:, h, :],
                             start=True, stop=True)
            nc.scalar.activation(dflat[:, h, :], dn[:], AF.Copy)
        dbt = fp1.tile([128, 2, CHUNK], BF16, tag="db", name=f"db_{c}")
        rbt = fp1.tile([128, 2, CHUNK], BF16, tag="rb", name=f"rb_{c}")
        nc.gpsimd.partition_broadcast(dbt[:], dflat[:])
        with nc.allow_low_precision(reason="bf16 softmax recip, tol 2e-2"):
            for h in range(HPC):
                nc.vector.reciprocal(rbt[:, h, :], dbt[:, h, :])
        # apply, transposed: acc += bcast(exp_i) * v^T[:, l - s_i]
        acc_a = fp.tile([128, 2, CHUNK], BF16, tag="acca", name=f"acca_{c}")
        acc_b = fp.tile([128, 2, CHUNK], BF16, tag="accb", name=f"accb_{c}")
        for w in range(8):
            ef = fp.tile([1, 2, 2, CHUNK], BF16, tag="eflat", name=f"ef_{c}_{w}")
            nc.sync.dma_start(ef[:], ec[2 * w:2 * w + 2, :, :])
            for j in range(2):
                i = 2 * w + j
                s = SEQ_SHIFTS[i]
                eb = fp.tile([128, 2, CHUNK], BF16, tag="eb", name=f"eb_{c}_{i}")
                nc.gpsimd.partition_broadcast(eb[:], ef[0:1, j, :, :])
                acc = acc_a if i < 8 else acc_b
                first = i in (0, 8)
                tgt = acc if first else fp.tile([128, 2, CHUNK], BF16,
                                                 tag="prod", name=f"ap_{c}_{i}")
                for (joff, srcl, rl) in _wrap_runs(w0 - s, CHUNK):
                    for h in range(HPC):
                        nc.vector.tensor_tensor(
                            tgt[:, h, joff:joff + rl],
                            eb[:, h, joff:joff + rl],
                            v_view[:, h, bL + srcl: bL + srcl + rl],
                            op=AluOp.mult)
                if not first:
                    for h in range(HPC):
                        nc.vector.tensor_tensor(acc[:, h, :], acc[:, h, :],
                                                tgt[:, h, :], op=AluOp.add)
        outT = fp.tile([128, 2, CHUNK], BF16, tag="qot", name=f"outT_{c}")
        for h in range(HPC):
            nc.vector.tensor_tensor(acc_a[:, h, :], acc_a[:, h, :],
                                    acc_b[:, h, :], op=AluOp.add)
            nc.vector.tensor_tensor(outT[:, h, :], acc_a[:, h, :],
                                    rbt[:, h, :], op=AluOp.mult)
        # output projection
        for t4 in range(4):
            t = c * 4 + t4
            for half in range(2):
                ost = fp.tile([128, D // 2], BF16, tag="ost", name=f"ost_{t}_{half}")
                for e2 in range(2):
                    e = half * 2 + e2
                    ops = p7ps.tile([128, 512], FP32, tag="p7", name=f"ops_{t}_{e}")
                    for dt in range(2):
                        nc.tensor.matmul(
                            ops[:],
                            outT[:, dt, 128 * t4:128 * (t4 + 1)],
                            wo_sb[:, dt * D + 512 * e: dt * D + 512 * (e + 1)],
                            start=(dt == 0), stop=(dt == 1))
                    nc.scalar.activation(ost[:, 512 * e2:512 * (e2 + 1)], ops[:],
                                         AF.Copy)
                nc.scalar.dma_start(
                    outp[128 * t:128 * (t + 1), half * (D // 2):(half + 1) * (D // 2)],
                    ost[:])

    # ---------------- schedule ----------------
    SPC = CHUNK // SUB  # subchunks per chunk = 2
    # batch 0 projections
    for c in range(8):
        for s in range(SPC):
            emit_p1_sub(c * SPC + s)
    emit_rms_half(0)
    # cover collective+rsqrt latency with the first batch-1 projections
    HEAD = 2
    for c in range(8, 8 + HEAD):
        for s in range(SPC):
            emit_p1_sub(c * SPC + s)
    # fused batch 0 interleaved with remaining batch 1 projections
    for idx in range(8):
        emit_fused(idx)
        if idx < 8 - HEAD:
            for s in range(SPC):
                emit_p1_sub((8 + HEAD + idx) * SPC + s)
    emit_rms_half(1)
    for idx in range(8, 16):
        emit_fused(idx)


_PROG = None


def _get_program():
    global _PROG
    if _PROG is None:
        _PROG = _build_program()
    return _PROG


def _host_prep(inputs):
    wq = np.asarray(inputs['wq'], np.float32)
    wk = np.asarray(inputs['wk'], np.float32)
    wv = np.asarray(inputs['wv'], np.float32)
    bq = np.asarray(inputs['bq'], np.float32)
    bk = np.asarray(inputs['bk'], np.float32)
    bv = np.asarray(inputs['bv'], np.float32)
    qnw = np.asarray(inputs['q_norm_w'], np.float32)
    knw = np.asarray(inputs['k_norm_w'], np.float32)
    mix = np.asarray(inputs['score_mix_w'], np.float32)[0]
    wo = np.asarray(inputs['wo'], np.float32)

    x = np.asarray(inputs['x'], np.float32)
    xT = np.ascontiguousarray(x.reshape(N, D).T).astype(BF)
    scale = DH ** -0.5

    oh = np.zeros((128, NS * 16), np.float32)
    for i in range(NS):
        oh[:, 16 * i + i] = 1.0
    oh = oh.astype(BF)
    ones16 = np.ones((16, 1), np.float32).astype(BF)

    in_maps = []
    for c in range(NCORES):
        cs = c * DPC
        sl = slice(cs, cs + DPC)
        wTc = np.concatenate([wq[sl].T, wk[sl].T, wv[sl].T], axis=1)  # [2048, 768]
        bias = np.stack([bq[cs:cs + 128], bq[cs + 128:cs + 256],
                         bk[cs:cs + 128], bk[cs + 128:cs + 256],
                         bv[cs:cs + 128], bv[cs + 128:cs + 256]], axis=1)
        pmT = np.zeros((128, HPC * 128), np.float32)
        for h in range(HPC):
            gh = c * HPC + h
            Pm = np.zeros((DH, DH), np.float32)
            for n, ch in enumerate([0] + CH_SHIFTS):
                for dd in range(DH):
                    dp = (dd - ch) % DH
                    Pm[dd, dp] += mix[n] * qnw[gh * DH + dp]
            Pm *= scale * knw[gh * DH:(gh + 1) * DH][:, None]
            pmT[:, 128 * h:128 * (h + 1)] = Pm.T
        woTc = np.ascontiguousarray(wo[:, sl].T)  # [256, 2048]
        in_maps.append({
            "xT": xT,
            "wT": wTc.astype(BF),
            "bias6": np.ascontiguousarray(bias),
            "pmT": pmT.astype(BF),
            "oh": oh,
            "ones16": ones16,
            "woT": woTc.astype(BF),
        })
    return in_maps


LAST_RESULT = None


def kernel(**inputs):
    global LAST_RESULT
    import os
    in_maps = _host_prep(inputs)
    nc = _get_program()
    trace = bool(os.environ.get("CRA_TRACE"))
    res = run_bass_kernel_spmd(nc, in_maps, list(range(NCORES)), trace=trace)
    LAST_RESULT = res
    acc = np.zeros((N, D), np.float32)
    for r in res.results:
        acc += np.asarray(r["outp"], np.float32)
    acc += np.asarray(inputs['bo'], np.float32)
    return acc.reshape(B, L, D)


# revision 8
# speedup vs baseline: 1.2956x; 1.1031x over previous
"""Trainium2 Bass kernel for nn_CliffordRollingAttention.

Strategy (head-parallel over 8 cores, 2 heads/core), v2 fused pipeline:
  - Host pre-transposes x -> xT [D, B*L] bf16, slices/folds weights per core.
  - On-device per core, per batch half:
      A: QKV projections on PE in transposed layout [d, l] (2 rotating PSUM
         banks, m-outer over 256-l subchunks), bf16 with fp32 PSUM.
         Per-row sumsq partials for RMS via ACT Square + PE ones-reduce.
      B: 64KB AllReduce of q/k sumsq partials, rsqrt -> bf16 rms rows.
      C (per 512-l chunk, fused): rms rows broadcast via gpsimd
         partition_broadcast; k-norm + qm build (PE matmul w/ host-folded
         mix matrix, DVE multiply); scores = DVE products qm*k[:, l-s]
         + PE one-hot reduce into [16, l] PSUM; ACT exp; denominator via
         PE ones-reduce + DVE reciprocal; exp rows flattened to partition 0
         (SBUF DMA) then gpsimd partition_broadcast to [128, l]; apply
         entirely in transposed layout: acc += eb_i * v^T[:, l-s] on DVE
         (v^T kept resident in SBUF, never transposed or respilled);
         normalize once by broadcast reciprocal; output projection directly
         from the transposed accumulator (PE), row-major PSUM -> bf16 out.
  - Emission interleaves batch-1 projections with batch-0 fused chunks so
    PE stays busy while DVE/GpSimd work, and vice versa.
  - Host sums the 8 partial outputs in fp32 and adds the output bias.
"""

import numpy as np
import ml_dtypes

import concourse.bass as bass
import concourse.bacc as bacc
import concourse.mybir as mybir
import concourse.tile as tile
from concourse import library_config
from concourse.bass_utils import run_bass_kernel_spmd

BF = ml_dtypes.bfloat16
FP32 = mybir.dt.float32
BF16 = mybir.dt.bfloat16

B, L, D = 2, 4096, 2048
H, DH = 16, 128
NCORES = 8
HPC = H // NCORES          # heads per core = 2
DPC = HPC * DH             # channels per core = 256
N = B * L                  # 8192 rows
EPS = 1e-6
SEQ_SHIFTS = [0, 1, -1, 3, -3, 9, -9, 26, -26, 78, -78, 232, -232, 689, -689, 2048]
CH_SHIFTS = [1, 2, 4, 8]
NS = len(SEQ_SHIFTS)       # 16
CHUNK = 512
NCHUNK = N // CHUNK        # 16
SUB = 256                  # P1 subchunk
AluOp = mybir.AluOpType
AF = mybir.ActivationFunctionType


def _wrap_runs(start, length):
    """Split output positions j in [0,length) whose source row is
    b*L + ((start + j) mod L) into maximal contiguous source runs.
    Returns list of (j_offset, src_local_start, run_len)."""
    runs = []
    j = 0
    while j < length:
        src = (start + j) % L
        run = min(length - j, L - src)
        runs.append((j, src, run))
        j += run
    return runs


def _aligned_runs(start, length):
    """_wrap_runs variant that forces every destination start (joff) even,
    so DVE writes are 4-byte aligned (odd starts cost ~4x on HW). An odd
    run start is expanded one element left (recomputing the previous run's
    last element, identical by modular arithmetic) unless the source would
    underflow, in which case a 1-element op is split off."""
    out = []
    for (joff, srcl, rl) in _wrap_runs(start, length):
        if joff % 2 == 1:
            if srcl >= 1:
                out.append((joff - 1, srcl - 1, rl + 1))
            else:
                out.append((joff, srcl, 1))
                if rl > 1:
                    out.append((joff + 1, srcl + 1, rl - 1))
        else:
            out.append((joff, srcl, rl))
    return out


def _build_program():
    nc = bacc.Bacc(num_devices=NCORES)

    handles = {
        "xT": nc.declare_dram_parameter("xT", [D, N], BF16, isOutput=False),
        "wT": nc.declare_dram_parameter("wT", [D, 6 * 128], BF16, isOutput=False),
        "bias6": nc.declare_dram_parameter("bias6", [128, 6], FP32, isOutput=False),
        "pmT": nc.declare_dram_parameter("pmT", [128, HPC * 128], BF16, isOutput=False),
        "oh": nc.declare_dram_parameter("oh", [128, NS * 16], BF16, isOutput=False),
        "ones16": nc.declare_dram_parameter("ones16", [16, 1], BF16, isOutput=False),
        "woT": nc.declare_dram_parameter("woT", [DPC, D], BF16, isOutput=False),
        "outp": nc.declare_dram_parameter("outp", [N, D], BF16, isOutput=True),
    }

    import contextlib
    with tile.TileContext(nc) as tc:
        with contextlib.ExitStack() as ctx:
            _emit_inner(ctx, tc, handles)
    nc.compile()
    return nc


def _emit_inner(ctx, tc, handles):
    nc = tc.nc
    xT = handles["xT"][:]
    wT = handles["wT"][:]
    bias6 = handles["bias6"][:]
    pmT_d = handles["pmT"][:]
    oh_d = handles["oh"][:]
    ones16_d = handles["ones16"][:]
    woT_d = handles["woT"][:]
    outp = handles["outp"][:]

    nc.gpsimd.load_library(library_config.proxy)

    # ---------------- persistent pools ----------------
    const = ctx.enter_context(tc.tile_pool(name="const", bufs=1))
    big = ctx.enter_context(tc.tile_pool(name="big", bufs=1))
    dram = ctx.enter_context(tc.tile_pool(name="dram", bufs=1, space="DRAM"))

    w_sb = const.tile([128, 16 * 768], BF16)        # 24KB
    bias_sb = const.tile([128, 6], FP32)
    pm_sb = const.tile([128, HPC * 128], BF16)
    oh_sb = const.tile([128, NS * 16], BF16)
    ones16_sb = const.tile([16, 1], BF16)
    eps_sb = const.tile([128, 1], FP32)
    wo_sb = const.tile([128, HPC * D], BF16)        # 8KB [128, dt*2048 + e]

    nc.sync.dma_start(w_sb[:].rearrange("p (k j) -> p k j", k=16),
                      wT.rearrange("(k p) j -> p k j", p=128))
    nc.sync.dma_start(bias_sb[:], bias6)
    nc.sync.dma_start(pm_sb[:], pmT_d)
    nc.sync.dma_start(oh_sb[:], oh_d)
    nc.sync.dma_start(ones16_sb[:], ones16_d)
    nc.gpsimd.memset(eps_sb[:], EPS)
    nc.sync.dma_start(wo_sb[:].rearrange("p (dt e) -> p dt e", dt=2),
                      woT_d.rearrange("(dt p) e -> p dt e", p=128))

    q_raw = big.tile([128, HPC * N], BF16)               # 32KB [p, h*N + l]
    k_sb = big.tile([128, HPC * N], BF16)                # 32KB
    v_sb = big.tile([128, HPC * N], BF16)                # 32KB

    q_view = q_raw[:].rearrange("p (h l) -> p h l", h=2)
    k_view = k_sb[:].rearrange("p (h l) -> p h l", h=2)
    v_view = v_sb[:].rearrange("p (h l) -> p h l", h=2)

    ss_dram = dram.tile([2, 2, N // 2], FP32)   # [half, q/k, l-in-half]
    ss_out = dram.tile([2, 2, N // 2], FP32)
    rms_dram = dram.tile([2, N], BF16)

    # ---------------- working pools ----------------
    p1x = ctx.enter_context(tc.tile_pool(name="p1x", bufs=2))
    sqp = ctx.enter_context(tc.tile_pool(name="sqp", bufs=2))
    fp = ctx.enter_context(tc.tile_pool(name="fp", bufs=2))
    fp1 = ctx.enter_context(tc.tile_pool(name="fp1", bufs=1))
    fpe = ctx.enter_context(tc.tile_pool(name="fpe", bufs=3))
    p1ps = ctx.enter_context(tc.tile_pool(name="p1ps", bufs=1, space="PSUM"))
    miscps = ctx.enter_context(tc.tile_pool(name="miscps", bufs=2, space="PSUM"))
    scps = ctx.enter_context(tc.tile_pool(name="scps", bufs=1, space="PSUM"))
    p7ps = ctx.enter_context(tc.tile_pool(name="p7ps", bufs=2, space="PSUM"))

    # ---------------- stage B: AllReduce + rsqrt ----------------
    def emit_rms_half(hf):
        HN = N // 2
        nc.gpsimd.collective_compute(
            "AllReduce", AluOp.add,
            replica_groups=[list(range(NCORES))],
            ins=[ss_dram[hf].opt()],
            outs=[ss_out[hf].opt()],
        )
        with tc.tile_pool(name=f"p2_{hf}", bufs=1) as p2:
            col = p2.tile([128, 64], FP32, name=f"col_{hf}")
            srt = p2.tile([128, 64], FP32, name=f"srt_{hf}")
            rinv = p2.tile([128, 64], BF16, name=f"rinv_{hf}")
            for r in range(2):
                nc.sync.dma_start(
                    col[:, r * 32:(r + 1) * 32],
                    ss_out[hf, r, :].rearrange("(t p) -> p t", p=128))
            nc.scalar.activation(srt[:], col[:], AF.Sqrt, bias=eps_sb[:],
                                 scale=1.0 / D)
            with nc.allow_low_precision(reason="bf16 rms factors, tol 2e-2"):
                nc.vector.reciprocal(rinv[:], srt[:])
            for r in range(2):
                nc.sync.dma_start(
                    rms_dram[r, hf * HN:(hf + 1) * HN]
                    .rearrange("(t p) -> p t", p=128),
                    rinv[:, r * 32:(r + 1) * 32])

    # ---------------- stage A: projections (one 256-l subchunk) ------------
    def emit_p1_sub(si):
        cs = si * SUB
        hfc, hcs = divmod(cs, N // 2)
        xt = p1x.tile([128, 16, SUB], BF16, tag="xt", name=f"xt_{si}")
        nc.sync.dma_start(xt[:],
                          xT[:, cs:cs + SUB].rearrange("(k p) l -> p k l", p=128))
        sqs = {}
        # m order: q0 q1 k0 k1 v0 v1
        for m in range(6):
            ps = p1ps.tile([128, SUB], FP32, tag=f"ps{m % 2}", name=f"ps{m}_{si}")
            for k in range(16):
                nc.tensor.matmul(
                    ps[:],
                    w_sb[:, k * 768 + 128 * m: k * 768 + 128 * (m + 1)],
                    xt[:, k, :],
                    start=(k == 0), stop=(k == 15),
                )
            kind, dt = divmod(m, 2)
            if kind == 0:    # q
                nc.scalar.activation(q_raw[:, dt * N + cs: dt * N + cs + SUB],
                                     ps[:], AF.Identity, bias=bias_sb[:, dt:dt + 1])
                sq = sqp.tile([128, SUB], BF16, tag=f"sq{dt}", name=f"sq{dt}_{si}")
                nc.scalar.activation(sq[:], ps[:], AF.Square,
                                     bias=bias_sb[:, dt:dt + 1])
                sqs[f"q{dt}"] = sq
            elif kind == 1:  # k
                nc.scalar.activation(k_sb[:, dt * N + cs: dt * N + cs + SUB],
                                     ps[:], AF.Identity, bias=bias_sb[:, 2 + dt:3 + dt])
                sq = sqp.tile([128, SUB], BF16, tag=f"sqk{dt}", name=f"sqk{dt}_{si}")
                nc.scalar.activation(sq[:], ps[:], AF.Square,
                                     bias=bias_sb[:, 2 + dt:3 + dt])
                sqs[f"k{dt}"] = sq
            else:            # v
                nc.scalar.activation(v_sb[:, dt * N + cs: dt * N + cs + SUB],
                                     ps[:], AF.Identity, bias=bias_sb[:, 4 + dt:5 + dt])
        # sumsq reduce via PE (ones column of oh), evict, ship to DRAM
        for kind, key in ((0, "q"), (1, "k")):
            ssq = miscps.tile([1, 512], FP32, tag="misc", name=f"ssq{key}_{si}")
            for dt in range(2):
                nc.tensor.matmul(ssq[:, 0:SUB], oh_sb[:, 0:1], sqs[f"{key}{dt}"][:],
                                 start=(dt == 0), stop=(dt == 1))
            ssr = sqp.tile([1, SUB], FP32, tag=f"ssr{key}", name=f"ssr{key}_{si}")
            nc.scalar.activation(ssr[:], ssq[:, 0:SUB], AF.Copy)
            nc.sync.dma_start(ss_dram[hfc, kind:kind + 1, hcs:hcs + SUB], ssr[:])

    _ec_tiles = {}
    _rcp_tiles = {}

    # ---------------- stage C: fused chunk (front: through exp/recip) -----
    def emit_front(c):
        cs = c * CHUNK
        b = cs // L
        w0 = cs - b * L
        bL = b * L
        # rms rows -> broadcast
        rq = fp1.tile([1, CHUNK], BF16, tag="rq", name=f"rq_{c}")
        rk = fp1.tile([1, CHUNK], BF16, tag="rk", name=f"rk_{c}")
        nc.sync.dma_start(rq[:], rms_dram[0:1, cs:cs + CHUNK])
        nc.sync.dma_start(rk[:], rms_dram[1:2, cs:cs + CHUNK])
        rqb = fp1.tile([128, CHUNK], BF16, tag="rqb", name=f"rqb_{c}")
        rkb = fp1.tile([128, CHUNK], BF16, tag="rkb", name=f"rkb_{c}")
        nc.gpsimd.partition_broadcast(rqb[:], rq[:])
        nc.gpsimd.partition_broadcast(rkb[:], rk[:])
        # k-norm
        for dt in range(2):
            nc.vector.tensor_tensor(k_view[:, dt, cs:cs + CHUNK],
                                    k_view[:, dt, cs:cs + CHUNK], rkb[:],
                                    op=AluOp.mult)
        # qm
        qmc = fp.tile([128, 2, CHUNK], BF16, tag="qot", name=f"qm_{c}")
        for h in range(HPC):
            qm_ps = miscps.tile([128, CHUNK], FP32, tag="misc", name=f"qmps_{c}_{h}")
            nc.tensor.matmul(qm_ps[:], pm_sb[:, 128 * h:128 * (h + 1)],
                             q_view[:, h, cs:cs + CHUNK], start=True, stop=True)
            nc.vector.tensor_tensor(qmc[:, h, :], qm_ps[:], rqb[:], op=AluOp.mult)
        # scores
        sc = scps.tile([16, 2, CHUNK], FP32, tag="sc", name=f"sc_{c}")
        for i, s in enumerate(SEQ_SHIFTS):
            pr = fp.tile([128, 2, CHUNK], BF16, tag="prod", name=f"pr_{c}_{i}")
            for (joff, srcl, rl) in _aligned_runs(w0 - s, CHUNK):
                for h in range(HPC):
                    nc.vector.tensor_tensor(
                        pr[:, h, joff:joff + rl],
                        qmc[:, h, joff:joff + rl],
                        k_view[:, h, bL + srcl: bL + srcl + rl],
                        op=AluOp.mult)
            for h in range(HPC):
                nc.tensor.matmul(sc[:, h, :], oh_sb[:, 16 * i:16 * (i + 1)],
                                 pr[:, h, :], start=(i == 0), stop=(i == NS - 1))
        # exp + den + reciprocal (exp(-ln(den)) keeps one act table)
        ec = fp.tile([16, 2, CHUNK], BF16, tag="expc", name=f"ec_{c}")
        _ec_tiles[c] = ec
        lnd = fp1.tile([1, 2, CHUNK], FP32, tag="lnd", name=f"lnd_{c}")
        rcp = fp.tile([1, 2, CHUNK], BF16, tag="rrow", name=f"rcp_{c}")
        _rcp_tiles[c] = rcp
        for h in range(HPC):
            nc.scalar.activation(ec[:, h, :], sc[:, h, :], AF.Exp)
        for h in range(HPC):
            dn = miscps.tile([1, CHUNK], FP32, tag="misc", name=f"dn_{c}_{h}")
            nc.tensor.matmul(dn[:], ones16_sb[:, 0:1], ec[:, h, :],
                             start=True, stop=True)
            nc.scalar.activation(lnd[:, h, :], dn[:], AF.Ln)
        with nc.allow_low_precision(reason="bf16 softmax recip, tol 2e-2"):
            nc.scalar.activation(rcp[:], lnd[:], AF.Exp, scale=-1.0)

    # ------------- stage C back half: broadcast, apply, out-projection -----
    def emit_back(c):
        cs = c * CHUNK
        b = cs // L
        w0 = cs - b * L
        bL = b * L
        ec = _ec_tiles[c]
        rcp = _rcp_tiles[c]
        rbt = fp1.tile([128, 2, CHUNK], BF16, tag="rb", name=f"rb_{c}")
        nc.gpsimd.partition_broadcast(rbt[:], rcp[:])
        acc_a = fp1.tile([128, 2, CHUNK], BF16, tag="acca", name=f"acca_{c}")
        acc_b = fp1.tile([128, 2, CHUNK], BF16, tag="accb", name=f"accb_{c}")
        efs = {}
        def flat(w):
            ef = fpe.tile([1, 2, 2, CHUNK], BF16, tag="eflat", name=f"ef_{c}_{w}")
            nc.sync.dma_start(ef[:], ec[2 * w:2 * w + 2, :, :])
            efs[w] = ef
        flat(0)
        flat(1)
        for w in range(8):
            if w + 2 < 8:
                flat(w + 2)
            ef = efs[w]
            for j in range(2):
                i = 2 * w + j
                s = SEQ_SHIFTS[i]
                eb = fp.tile([128, 2, CHUNK], BF16, tag="eb", name=f"eb_{c}_{i}")
                nc.gpsimd.partition_broadcast(eb[:], ef[0:1, j, :, :])
                acc = acc_a if i < 8 else acc_b
                first = i in (0, 8)
                tgt = acc if first else fp.tile([128, 2, CHUNK], BF16,
                                                tag="prod", name=f"ap_{c}_{i}")
                for (joff, srcl, rl) in _aligned_runs(w0 - s, CHUNK):
                    for h in range(HPC):
                        nc.vector.tensor_tensor(
                            tgt[:, h, joff:joff + rl],
                            eb[:, h, joff:joff + rl],
                            v_view[:, h, bL + srcl: bL + srcl + rl],
                            op=AluOp.mult)
                if not first:
                    for h in range(HPC):
                        nc.vector.tensor_tensor(acc[:, h, :], acc[:, h, :],
                                                tgt[:, h, :], op=AluOp.add)
        outT = fp.tile([128, 2, CHUNK], BF16, tag="qot", name=f"outT_{c}")
        for h in range(HPC):
            nc.vector.tensor_tensor(acc_a[:, h, :], acc_a[:, h, :],
                                    acc_b[:, h, :], op=AluOp.add)
            nc.vector.tensor_tensor(outT[:, h, :], acc_a[:, h, :],
                                    rbt[:, h, :], op=AluOp.mult)
        # output projection
        for t4 in range(4):
            t = c * 4 + t4
            for half in range(2):
                ost = fp.tile([128, D // 2], BF16, tag="ost", name=f"ost_{t}_{half}")
                for e2 in range(2):
                    e = half * 2 + e2
                    ops = p7ps.tile([128, 512], FP32, tag="p7", name=f"ops_{t}_{e}")
                    for dt in range(2):
                        nc.tensor.matmul(
                            ops[:],
                            outT[:, dt, 128 * t4:128 * (t4 + 1)],
                            wo_sb[:, dt * D + 512 * e: dt * D + 512 * (e + 1)],
                            start=(dt == 0), stop=(dt == 1))
                    nc.scalar.activation(ost[:, 512 * e2:512 * (e2 + 1)], ops[:],
                                         AF.Copy)
                nc.scalar.dma_start(
                    outp[128 * t:128 * (t + 1), half * (D // 2):(half + 1) * (D // 2)],
                    ost[:])

    # ---------------- schedule ----------------
    SPC = CHUNK // SUB  # subchunks per chunk = 2

    def p1c(c):
        for s in range(SPC):
            emit_p1_sub(c * SPC + s)

    # batch 0 projections
    for c in range(8):
        p1c(c)
    emit_rms_half(0)
    # cover collective latency with first batch-1 projections
    p1c(8)
    p1c(9)
    # software-pipelined fused chunks: front(c+1) before back(c)
    emit_front(0)
    for idx in range(7):
        emit_front(idx + 1)
        emit_back(idx)
        if idx < 6:
            p1c(10 + idx)
    emit_rms_half(1)
    emit_back(7)
    emit_front(8)
    for idx in range(8, 15):
        emit_front(idx + 1)
        emit_back(idx)
    emit_back(15)


_PROG = None
def _get_program():
    global _PROG
    if _PROG is None:
        _PROG = _build_program()
    return _PROG


def _host_prep(inputs):
    wq = np.asarray(inputs['wq'], np.float32)
    wk = np.asarray(inputs['wk'], np.float32)
    wv = np.asarray(inputs['wv'], np.float32)
    bq = np.asarray(inputs['bq'], np.float32)
    bk = np.asarray(inputs['bk'], np.float32)
    bv = np.asarray(inputs['bv'], np.float32)
    qnw = np.asarray(inputs['q_norm_w'], np.float32)
    knw = np.asarray(inputs['k_norm_w'], np.float32)
    mix = np.asarray(inputs['score_mix_w'], np.float32)[0]
    wo = np.asarray(inputs['wo'], np.float32)

    x = np.asarray(inputs['x'], np.float32)
    xT = np.ascontiguousarray(x.reshape(N, D).T).astype(BF)
    scale = DH ** -0.5

    oh = np.zeros((128, NS * 16), np.float32)
    for i in range(NS):
        oh[:, 16 * i + i] = 1.0
    oh = oh.astype(BF)
    ones16 = np.ones((16, 1), np.float32).astype(BF)

    in_maps = []
    for c in range(NCORES):
        cs = c * DPC
        sl = slice(cs, cs + DPC)
        wTc = np.concatenate([wq[sl].T, wk[sl].T, wv[sl].T], axis=1)  # [2048, 768]
        bias = np.stack([bq[cs:cs + 128], bq[cs + 128:cs + 256],
                         bk[cs:cs + 128], bk[cs + 128:cs + 256],
                         bv[cs:cs + 128], bv[cs + 128:cs + 256]], axis=1)
        pmT = np.zeros((128, HPC * 128), np.float32)
        for h in range(HPC):
            gh = c * HPC + h
            Pm = np.zeros((DH, DH), np.float32)
            for n, ch in enumerate([0] + CH_SHIFTS):
                for dd in range(DH):
                    dp = (dd - ch) % DH
                    Pm[dd, dp] += mix[n] * qnw[gh * DH + dp]
            Pm *= scale * knw[gh * DH:(gh + 1) * DH][:, None]
            pmT[:, 128 * h:128 * (h + 1)] = Pm.T
        woTc = np.ascontiguousarray(wo[:, sl].T)  # [256, 2048]
        in_maps.append({
            "xT": xT,
            "wT": wTc.astype(BF),
            "bias6": np.ascontiguousarray(bias),
            "pmT": pmT.astype(BF),
            "oh": oh,
            "ones16": ones16,
            "woT": woTc.astype(BF),
        })
    return in_maps


LAST_RESULT = None


def kernel(**inputs):
    global LAST_RESULT
    import os
    in_maps = _host_prep(inputs)
    nc = _get_program()
    trace = bool(os.environ.get("CRA_TRACE"))
    res = run_bass_kernel_spmd(nc, in_maps, list(range(NCORES)), trace=trace)
    LAST_RESULT = res
    acc = np.zeros((N, D), np.float32)
    for r in res.results:
        acc += np.asarray(r["outp"], np.float32)
    acc += np.asarray(inputs['bo'], np.float32)
    return acc.reshape(B, L, D)
